# revision 19
# baseline (speedup 1.0000x reference)
"""Trainium2 kernel for nn_BalancedHamiltonLayer.

Math: out = einsum("btd,rde->bte", x, factors)/sqrt(rank) + bias.
The einsum contracts r as a plain sum, so sum_r (x @ F_r) == x @ (sum_r F_r):
one [16384,2048] @ [2048,2048] GEMM instead of eight.

Distribution over 8 NeuronCores (single SPMD program), DP2 x TP4:
  - core c = (dp, tp) with dp = c//4, tp = c%4 owns output rows
    [dp*8192, (dp+1)*8192) and output cols [tp*512, (tp+1)*512).
  - W reduction is split between the two cores sharing a tp column group:
    core (dp, tp) reduces cols [tp*512 + dp*256, +256) from its factor
    slice (bf16 strided-pair adds on DVE: 3 instructions per 128-col
    chunk instead of a 7-add tree), then one tiny pairwise AllGather
    (1 MB, groups [[0,4],[1,5],[2,6],[3,7]]) swaps the halves.
  - SPMD slot trick: every core computes W_peer = (slot0 + slot1) -
    W_local in fp32 (exact: W_local equals one slot bitwise, so the
    subtraction recovers the other slot's bf16 values exactly). This
    keeps the program identical across cores with no per-core indexing.
  - GEMM in bf16 (fp32 PSUM): phase A = first PRE m-tiles against the
    locally reduced cols (runs in the AllGather's shadow), phase B =
    same m-tiles against peer cols, phase C = remaining m-tiles against
    all four 128-col e-tiles. 1/sqrt(8) scale + bias fused into the
    PSUM eviction (scalar_tensor_tensor on DVE).
  - Each core writes out^T [512e, 8192m] fp32; the host transposes back.

Host side shards/lays out inputs (partition-major swizzle, bf16 cast) and
reassembles the per-core outputs.
"""

import math

import numpy as np

B, T, DIM, RANK = 4, 4096, 2048, 8
N_CORES = 8
DP, TP = 2, 4
MC = (B * T) // DP             # 8192 rows per core
ECO = DIM // TP                # 512 output cols per core
ECL = ECO // DP                # 256 cols reduced locally
NT = DIM // 128                # 16 contraction tiles
NMJ = MC // 512                # 16 m-supertiles per core
PRE = 6                        # m-tiles run split (local cols first)
SCALE = 1.0 / math.sqrt(RANK)

_CACHE = {}


def _build():
    import concourse.bacc as bacc
    import concourse.mybir as mybir
    import concourse.tile as tile

    f32 = mybir.dt.float32
    bf16 = mybir.dt.bfloat16
    add = mybir.AluOpType.add
    mult = mybir.AluOpType.mult
    pair_groups = [[2 * g, 2 * g + 1] for g in range(4)]

    nc = bacc.Bacc(
        "TRN2", target_bir_lowering=False, debug=False, num_devices=N_CORES
    )
    # x^T tiles: d = t*128+p, m_global = dp*8192 + mj*512 + m
    xh = nc.dram_tensor("xh", [NMJ, 128, NT, 512], bf16, kind="ExternalInput").ap()
    # local factor cols: [ec, p, rp, q, t, e]; r = rp*2+q, d = t*128+p,
    # e_global = tp*512 + dp*256 + ec*128 + e
    fh = nc.dram_tensor(
        "fh", [2, 128, RANK // 2, 2, NT, 128], bf16, kind="ExternalInput"
    ).ap()
    # bias per (p, slot) replicated over m; slot order [L0, L1, P0, P1]
    bias_melt = nc.dram_tensor(
        "bias_melt", [128, 4, 512], f32, kind="ExternalInput"
    ).ap()
    # transposed output: rows = [L0 L1 P0 P1] col chunks, cols = m
    outT = nc.dram_tensor("outT", [ECO, MC], f32, kind="ExternalOutput").ap()

    with tile.TileContext(nc) as tc:
        with (
            tc.tile_pool(name="const", bufs=1) as const_pool,
            tc.tile_pool(name="dram", bufs=1, space="DRAM") as dram_pool,
            tc.tile_pool(name="wsb", bufs=1) as w_pool,
            tc.tile_pool(name="xa", bufs=6) as x_pool,
        ):
            scope = nc.named_scope
            xsb = [x_pool.tile([128, NT, 512], bf16, tag="x", name=f"x{i}")
                   for i in range(NMJ)]
            # Factor loads + bias enqueue FIRST: the physical DMA queues
            # drain roughly in enqueue order, so anything issued before
            # them adds straight latency to the reduction -> AllGather ->
            # peer-W critical path.
            ftiles = []
            with tc.tile_pool(name="red", bufs=1) as red_pool:
                for ec in range(2):
                    ft = red_pool.tile([128, RANK // 2, 2, NT, 128], bf16,
                                       tag=f"fr{ec}", name=f"f{ec}")
                    (nc.sync if ec == 0 else nc.scalar).dma_start(ft[:], fh[ec])
                    ftiles.append(ft)
                bias_sb = const_pool.tile([128, 4, 512], f32)
                with tc.tile_wait_until(0.016):
                    nc.scalar.dma_start(bias_sb[:], bias_melt[:])

                # First two x tiles on gpsimd, timestamp-staggered: the
                # 16 physical DMA queues round-robin all rings, so an eager
                # x burst fair-shares against the factor loads / wc stores /
                # AllGather payloads and delays the whole critical path.
                # x2..x5 are issued after the collectives so the AllGather
                # triggers sit early in the gpsimd DGE ring (a full ring
                # blocks the sequencer and with it the collective dispatch).
                for mj in range(2):
                    with tc.tile_wait_until(0.015 + 0.006 * mj):
                        nc.gpsimd.dma_start(xsb[mj][:], xh[mj])

                wcs, w_pairs = [], []
                for ec in range(2):
                    wcs.append(dram_pool.tile([128, NT, 128], bf16,
                                              name=f"wc{ec}"))
                    w_pairs.append(dram_pool.tile([2, 128, NT, 128], bf16,
                                                  name=f"w_pair{ec}"))

                # Phase R: local W chunks; 3 strided-pair adds per chunk on
                # DVE (bf16 2x mode); each chunk's pairwise AllGather fires
                # as soon as that chunk's store lands. x2..x5 triggers are
                # interleaved so AG1's input-semaphore wait (which blocks
                # the gpsimd sequencer) never delays an x load.
                wl = w_pool.tile([128, 2, NT, 128], bf16)
                for ec in range(2):
                    with scope(f"reduce{ec}"):
                        s1 = red_pool.tile([128, RANK // 2, NT, 128], bf16,
                                           tag="s1")
                        nc.vector.tensor_add(s1[:], ftiles[ec][:, :, 0],
                                             ftiles[ec][:, :, 1])
                        s2 = red_pool.tile([128, 2, NT, 128], bf16, tag="s2")
                        nc.vector.tensor_add(s2[:], s1[:, 0:2], s1[:, 2:4])
                        nc.vector.tensor_add(wl[:, ec], s2[:, 0], s2[:, 1])
                        nc.sync.dma_start(wcs[ec][:], wl[:, ec])
                    with scope(f"ag{ec}"):
                        nc.gpsimd.collective_compute(
                            "AllGather", mybir.AluOpType.bypass,
                            ins=[wcs[ec].opt()],
                            outs=[w_pairs[ec].opt()],
                            replica_groups=pair_groups,
                        )
                    for mj in (2, 3) if ec == 0 else (4, 5):
                        with tc.tile_wait_until(0.015 + 0.006 * mj):
                            nc.gpsimd.dma_start(xsb[mj][:], xh[mj])

            # Peer W inputs: all on the scalar ring so the (late-firing)
            # triggers never block evict stores, which all go on sync.
            # Separate pool opened after red closes, so the factor tiles
            # and the peer-W tiles never need SBUF space simultaneously.
            wpx_ctx = tc.tile_pool(name="wpx", bufs=1)
            wpx_pool = wpx_ctx.__enter__()
            wp = [wpx_pool.tile([128, 2, NT, 128], bf16, name=f"wp{ec}")
                  for ec in range(2)]
            for ec in range(2):
                for s in range(2):
                    nc.scalar.dma_start(wp[ec][:, s], w_pairs[ec][s])
            wsum = wpx_pool.tile([128, NT, 128], f32)
            wpeer = wpx_pool.tile([128, 2, NT, 128], bf16)

            # slot0 + slot1 - W_local == the peer chunk, exactly (W_local
            # is bitwise one of the slots). On gpsimd: slower than DVE but
            # entirely off the DVE eviction stream, so the scheduler cannot
            # order it ahead of phase-A evictions (psum backpressure).
            for ec in range(2):
                with scope(f"peer{ec}"):
                    nc.gpsimd.tensor_add(wsum[:], wp[ec][:, 0], wp[ec][:, 1])
                    nc.gpsimd.tensor_sub(wpeer[:, ec], wsum[:], wl[:, ec])

            def sweep(ps_slice, w_ec, xt):
                for t in range(NT):
                    nc.tensor.matmul(
                        ps_slice,
                        w_ec[:, t, :],
                        xt[:, t, :],
                        start=(t == 0),
                        stop=(t == NT - 1),
                    )

            def evict(ps_slice, bias_slice, osb, mj, r0):
                nc.vector.scalar_tensor_tensor(
                    osb[:], ps_slice, SCALE, bias_slice, mult, add
                )
                for k in range(osb.shape[1]):
                    nc.sync.dma_start(
                        outT[(r0 + k) * 128:(r0 + k + 1) * 128,
                             mj * 512:(mj + 1) * 512],
                        osb[:, k],
                    )

            def prefetch(i):
                if i < NMJ:
                    nc.gpsimd.dma_start(xsb[i][:], xh[i])

            with (
                tc.tile_pool(name="osb", bufs=2) as o_pool,
                tc.tile_pool(name="ps", bufs=2, space="PSUM") as p_pool,
            ):
                # Phase A: first PRE m-tiles x local cols (AllGather shadow).
                # peer_ops(0) is slotted into the DVE stream near the end of
                # A: late enough not to head-of-line block A's evictions
                # behind the AllGather, early enough to unblock phase B0.
                for mj in range(PRE):
                    with scope(f"gA{mj}"):
                        ps = p_pool.tile([128, 4, 512], f32, tag="ps")
                        sweep(ps[:, 0, :], wl[:, 0], xsb[mj])
                        sweep(ps[:, 1, :], wl[:, 1], xsb[mj])
                        osb = o_pool.tile([128, 2, 512], f32, tag="osb")
                        evict(ps[:, 0:2], bias_sb[:, 0:2], osb, mj, 0)
                # Phase B0: same m-tiles x first peer chunk (single-sweep
                # units; the L/P column blocks are independent outputs, so
                # the two peer chunks can land as separate passes).
                for mj in range(PRE):
                    with scope(f"gB0_{mj}"):
                        ps = p_pool.tile([128, 4, 512], f32, tag="ps")
                        sweep(ps[:, 0, :], wpeer[:, 0], xsb[mj])
                        osb = o_pool.tile([128, 1, 512], f32, tag="osb")
                        evict(ps[:, 0:1], bias_sb[:, 2:3], osb, mj, 2)
                # Phase B1: same m-tiles x second peer chunk.
                for mj in range(PRE):
                    with scope(f"gB1_{mj}"):
                        ps = p_pool.tile([128, 4, 512], f32, tag="ps")
                        sweep(ps[:, 0, :], wpeer[:, 1], xsb[mj])
                        prefetch(PRE + mj)
                        osb = o_pool.tile([128, 1, 512], f32, tag="osb")
                        evict(ps[:, 0:1], bias_sb[:, 3:4], osb, mj, 3)
                # Phase C: remaining m-tiles x all four e-tiles.
                for mj in range(PRE, NMJ):
                    with scope(f"gC{mj}"):
                        ps = p_pool.tile([128, 4, 512], f32, tag="ps")
                        sweep(ps[:, 0, :], wl[:, 0], xsb[mj])
                        sweep(ps[:, 1, :], wl[:, 1], xsb[mj])
                        sweep(ps[:, 2, :], wpeer[:, 0], xsb[mj])
                        sweep(ps[:, 3, :], wpeer[:, 1], xsb[mj])
                        prefetch(mj + PRE)
                        osb = o_pool.tile([128, 4, 512], f32, tag="osb")
                        evict(ps[:], bias_sb[:], osb, mj, 0)
            wpx_ctx.__exit__(None, None, None)

    nc.compile()
    return nc


def _get_nc():
    if "nc" not in _CACHE:
        _CACHE["nc"] = _build()
    return _CACHE["nc"]


def _shard(x, factors, bias):
    import ml_dtypes

    bf = ml_dtypes.bfloat16
    x_flat = np.asarray(x, dtype=np.float32).reshape(B * T, DIM).astype(bf)
    factors = np.asarray(factors, dtype=np.float32).astype(bf)
    bias = np.ascontiguousarray(bias, dtype=np.float32)
    in_maps = []
    for c in range(N_CORES):
        tp, dp = c // DP, c % DP
        xc = x_flat[dp * MC:(dp + 1) * MC, :]           # [m, d]
        # -> [mj, p, t, m] with d = t*128+p, m = mj*512+m'
        xh = np.ascontiguousarray(
            xc.T.reshape(NT, 128, NMJ, 512).transpose(2, 1, 0, 3)
        )
        c0 = tp * ECO + dp * ECL
        fc = factors[:, :, c0:c0 + ECL]                 # [r, d, e]
        # -> [ec, p, rp, q, t, e]
        fhc = np.ascontiguousarray(
            fc.reshape(RANK // 2, 2, NT, 128, 2, 128).transpose(4, 3, 0, 1, 2, 5)
        )
        colmap = [tp * ECO + dp * ECL, tp * ECO + dp * ECL + 128,
                  tp * ECO + (1 - dp) * ECL, tp * ECO + (1 - dp) * ECL + 128]
        b4 = np.stack([bias[cm:cm + 128] for cm in colmap], axis=1)  # [128, 4]
        bias_melt = np.ascontiguousarray(
            np.broadcast_to(b4[:, :, None], (128, 4, 512)), dtype=np.float32
        )
        in_maps.append({"xh": xh, "fh": fhc, "bias_melt": bias_melt})
    return in_maps


def _run(in_maps, trace=False, trace_cores=None):
    from concourse.bass_utils import run_bass_kernel_spmd

    nc = _get_nc()
    return run_bass_kernel_spmd(
        nc, in_maps, list(range(N_CORES)), trace=trace, trace_cores=trace_cores
    )


def _assemble(res):
    out = np.empty((B * T, DIM), dtype=np.float32)
    for c in range(N_CORES):
        tp, dp = c // DP, c % DP
        outT = res.results[c]["outT"]
        colmap = [tp * ECO + dp * ECL, tp * ECO + dp * ECL + 128,
                  tp * ECO + (1 - dp) * ECL, tp * ECO + (1 - dp) * ECL + 128]
        for k, cm in enumerate(colmap):
            out[dp * MC:(dp + 1) * MC, cm:cm + 128] = \
                outT[k * 128:(k + 1) * 128, :].T
    return out.reshape(B, T, DIM)


def kernel(x, factors, bias):
    res = _run(_shard(x, factors, bias), trace=False)
    return _assemble(res)


# revision 20
# speedup vs baseline: 1.1200x; 1.1200x over previous
"""Trainium2 kernel for nn_BalancedHamiltonLayer.

Math: out = einsum("btd,rde->bte", x, factors)/sqrt(rank) + bias.
The einsum contracts r as a plain sum, so sum_r (x @ F_r) == x @ (sum_r F_r):
one [16384,2048] @ [2048,2048] GEMM instead of eight.

Distribution over 8 NeuronCores (single SPMD program), DP2 x TP4:
  - core c = (dp, tp) with dp = c//4, tp = c%4 owns output rows
    [dp*8192, (dp+1)*8192) and output cols [tp*512, (tp+1)*512).
  - W reduction is split between the two cores sharing a tp column group:
    core (dp, tp) reduces cols [tp*512 + dp*256, +256) from its factor
    slice (bf16 strided-pair adds on DVE: 3 instructions per 128-col
    chunk instead of a 7-add tree), then one tiny pairwise AllGather
    (1 MB, groups [[0,4],[1,5],[2,6],[3,7]]) swaps the halves.
  - SPMD slot trick: every core computes W_peer = (slot0 + slot1) -
    W_local in fp32 (exact: W_local equals one slot bitwise, so the
    subtraction recovers the other slot's bf16 values exactly). This
    keeps the program identical across cores with no per-core indexing.
  - GEMM in bf16 (fp32 PSUM): phase A = first PRE m-tiles against the
    locally reduced cols (runs in the AllGather's shadow), phase B =
    same m-tiles against peer cols, phase C = remaining m-tiles against
    all four 128-col e-tiles. 1/sqrt(8) scale + bias fused into the
    PSUM eviction (scalar_tensor_tensor on DVE).
  - Each core writes out^T [512e, 8192m] fp32; the host transposes back.

Host side shards/lays out inputs (partition-major swizzle, bf16 cast) and
reassembles the per-core outputs.
"""

import math

import numpy as np

B, T, DIM, RANK = 4, 4096, 2048, 8
N_CORES = 8
DP, TP = 2, 4
MC = (B * T) // DP             # 8192 rows per core
ECO = DIM // TP                # 512 output cols per core
ECL = ECO // DP                # 256 cols reduced locally
NT = DIM // 128                # 16 contraction tiles
NMJ = MC // 512                # 16 m-supertiles per core
PRE = 6                        # m-tiles run split (local cols first)
SCALE = 1.0 / math.sqrt(RANK)

_CACHE = {}


def _build():
    import concourse.bacc as bacc
    import concourse.mybir as mybir
    import concourse.tile as tile

    f32 = mybir.dt.float32
    bf16 = mybir.dt.bfloat16
    add = mybir.AluOpType.add
    mult = mybir.AluOpType.mult
    pair_groups = [[2 * g, 2 * g + 1] for g in range(4)]

    nc = bacc.Bacc(
        "TRN2", target_bir_lowering=False, debug=False, num_devices=N_CORES
    )
    # x^T tiles: d = t*128+p, m_global = dp*8192 + mj*512 + m
    xh = nc.dram_tensor("xh", [NMJ, 128, NT, 512], bf16, kind="ExternalInput").ap()
    # local factor cols: [ec, p, rp, q, t, e]; r = rp*2+q, d = t*128+p,
    # e_global = tp*512 + dp*256 + ec*128 + e
    fh = nc.dram_tensor(
        "fh", [2, 128, RANK // 2, 2, NT, 128], bf16, kind="ExternalInput"
    ).ap()
    # bias per (p, slot) replicated over m; slot order [L0, L1, P0, P1]
    bias_melt = nc.dram_tensor(
        "bias_melt", [128, 4, 512], f32, kind="ExternalInput"
    ).ap()
    # transposed output: rows = [L0 L1 P0 P1] col chunks, cols = m
    outT = nc.dram_tensor("outT", [ECO, MC], f32, kind="ExternalOutput").ap()

    with tile.TileContext(nc) as tc:
        with (
            tc.tile_pool(name="const", bufs=1) as const_pool,
            tc.tile_pool(name="dram", bufs=1, space="DRAM") as dram_pool,
            tc.tile_pool(name="wsb", bufs=1) as w_pool,
            tc.tile_pool(name="xa", bufs=6) as x_pool,
        ):
            scope = nc.named_scope
            xsb = [x_pool.tile([128, NT, 512], bf16, tag="x", name=f"x{i}")
                   for i in range(NMJ)]
            # Factor loads + bias enqueue FIRST: the physical DMA queues
            # drain roughly in enqueue order, so anything issued before
            # them adds straight latency to the reduction -> AllGather ->
            # peer-W critical path.
            # Factor loads: 8.4 MB per core dominates the front-end, so
            # each 4.2 MB e-chunk is split into two rank-pair halves loaded
            # on BOTH rings in parallel, and the reduction tree consumes
            # each half as it lands. f0 gets the rings first; everything
            # else (x tiles, bias) is staggered in behind it.
            with tc.tile_pool(name="red", bufs=1) as red_pool:
                fhalves = {}
                for ec in range(2):
                    for h in range(2):
                        ft = red_pool.tile([128, 2, 2, NT, 128], bf16,
                                           tag=f"fr{ec}{h}", name=f"f{ec}{h}")
                        (nc.sync if h == 0 else nc.scalar).dma_start(
                            ft[:], fh[ec, :, 2 * h:2 * h + 2])
                        fhalves[ec, h] = ft
                bias_sb = const_pool.tile([128, 4, 512], f32)
                nc.scalar.dma_start(bias_sb[:], bias_melt[:])

                # x tiles on gpsimd, staggered behind the factor stream.
                for mj in range(2):
                    with tc.tile_wait_until(0.012 + 0.006 * mj):
                        nc.gpsimd.dma_start(xsb[mj][:], xh[mj])

                wcs, w_pairs = [], []
                for ec in range(2):
                    wcs.append(dram_pool.tile([128, NT, 128], bf16,
                                              name=f"wc{ec}"))
                    w_pairs.append(dram_pool.tile([2, 128, NT, 128], bf16,
                                                  name=f"w_pair{ec}"))

                # Phase R: per half: s_h = (f_h[:,0]+f_h[:,1]) reduced again;
                # wl[ec] = s_0 + s_1. Each chunk's pairwise AllGather fires
                # as soon as its store lands. x triggers interleave so AG1's
                # input-semaphore wait never delays an x load.
                wl = w_pool.tile([128, 2, NT, 128], bf16)
                for ec in range(2):
                    with scope(f"reduce{ec}"):
                        s2 = red_pool.tile([128, 2, NT, 128], bf16, tag="s2")
                        for h in range(2):
                            ft = fhalves[ec, h]
                            s1 = red_pool.tile([128, 2, NT, 128], bf16,
                                               tag=f"s1{h}")
                            nc.vector.tensor_add(s1[:], ft[:, 0], ft[:, 1])
                            nc.vector.tensor_add(s2[:, h], s1[:, 0], s1[:, 1])
                        nc.vector.tensor_add(wl[:, ec], s2[:, 0], s2[:, 1])
                        nc.sync.dma_start(wcs[ec][:], wl[:, ec])
                    with scope(f"ag{ec}"):
                        nc.gpsimd.collective_compute(
                            "AllGather", mybir.AluOpType.bypass,
                            ins=[wcs[ec].opt()],
                            outs=[w_pairs[ec].opt()],
                            replica_groups=pair_groups,
                        )
                    for mj in (2, 3) if ec == 0 else (4, 5):
                        with tc.tile_wait_until(0.012 + 0.0065 * mj):
                            nc.gpsimd.dma_start(xsb[mj][:], xh[mj])

            # Peer W inputs: all on the scalar ring so the (late-firing)
            # triggers never block evict stores, which all go on sync.
            # Separate pool opened after red closes, so the factor tiles
            # and the peer-W tiles never need SBUF space simultaneously.
            wpx_ctx = tc.tile_pool(name="wpx", bufs=1)
            wpx_pool = wpx_ctx.__enter__()
            wp = [wpx_pool.tile([128, 2, NT, 128], bf16, name=f"wp{ec}")
                  for ec in range(2)]
            for ec in range(2):
                for s in range(2):
                    nc.scalar.dma_start(wp[ec][:, s], w_pairs[ec][s])
            wsum = wpx_pool.tile([128, NT, 128], f32)
            wpeer = wpx_pool.tile([128, 2, NT, 128], bf16)

            # slot0 + slot1 - W_local == the peer chunk, exactly (W_local
            # is bitwise one of the slots). On gpsimd: slower than DVE but
            # entirely off the DVE eviction stream, so the scheduler cannot
            # order it ahead of phase-A evictions (psum backpressure).
            for ec in range(2):
                with scope(f"peer{ec}"):
                    nc.gpsimd.tensor_add(wsum[:], wp[ec][:, 0], wp[ec][:, 1])
                    nc.gpsimd.tensor_sub(wpeer[:, ec], wsum[:], wl[:, ec])

            def sweep(ps_slice, w_ec, xt):
                for t in range(NT):
                    nc.tensor.matmul(
                        ps_slice,
                        w_ec[:, t, :],
                        xt[:, t, :],
                        start=(t == 0),
                        stop=(t == NT - 1),
                    )

            def evict(ps_slice, bias_slice, osb, mj, r0):
                nc.vector.scalar_tensor_tensor(
                    osb[:], ps_slice, SCALE, bias_slice, mult, add
                )
                for k in range(osb.shape[1]):
                    nc.sync.dma_start(
                        outT[(r0 + k) * 128:(r0 + k + 1) * 128,
                             mj * 512:(mj + 1) * 512],
                        osb[:, k],
                    )

            def prefetch(i):
                if i < NMJ:
                    nc.gpsimd.dma_start(xsb[i][:], xh[i])

            with (
                tc.tile_pool(name="osb", bufs=2) as o_pool,
                tc.tile_pool(name="ps", bufs=2, space="PSUM") as p_pool,
            ):
                # Phase A: first PRE m-tiles x local cols (AllGather shadow).
                # peer_ops(0) is slotted into the DVE stream near the end of
                # A: late enough not to head-of-line block A's evictions
                # behind the AllGather, early enough to unblock phase B0.
                for mj in range(PRE):
                    with scope(f"gA{mj}"):
                        ps = p_pool.tile([128, 4, 512], f32, tag="ps")
                        sweep(ps[:, 0, :], wl[:, 0], xsb[mj])
                        sweep(ps[:, 1, :], wl[:, 1], xsb[mj])
                        osb = o_pool.tile([128, 2, 512], f32, tag="osb")
                        evict(ps[:, 0:2], bias_sb[:, 0:2], osb, mj, 0)
                # Phase B0: same m-tiles x first peer chunk (single-sweep
                # units; the L/P column blocks are independent outputs, so
                # the two peer chunks can land as separate passes).
                for mj in range(PRE):
                    with scope(f"gB0_{mj}"):
                        ps = p_pool.tile([128, 4, 512], f32, tag="ps")
                        sweep(ps[:, 0, :], wpeer[:, 0], xsb[mj])
                        osb = o_pool.tile([128, 1, 512], f32, tag="osb")
                        evict(ps[:, 0:1], bias_sb[:, 2:3], osb, mj, 2)
                # Phase B1: same m-tiles x second peer chunk.
                for mj in range(PRE):
                    with scope(f"gB1_{mj}"):
                        ps = p_pool.tile([128, 4, 512], f32, tag="ps")
                        sweep(ps[:, 0, :], wpeer[:, 1], xsb[mj])
                        prefetch(PRE + mj)
                        osb = o_pool.tile([128, 1, 512], f32, tag="osb")
                        evict(ps[:, 0:1], bias_sb[:, 3:4], osb, mj, 3)
                # Phase C: remaining m-tiles x all four e-tiles.
                for mj in range(PRE, NMJ):
                    with scope(f"gC{mj}"):
                        ps = p_pool.tile([128, 4, 512], f32, tag="ps")
                        sweep(ps[:, 0, :], wl[:, 0], xsb[mj])
                        sweep(ps[:, 1, :], wl[:, 1], xsb[mj])
                        sweep(ps[:, 2, :], wpeer[:, 0], xsb[mj])
                        sweep(ps[:, 3, :], wpeer[:, 1], xsb[mj])
                        prefetch(mj + PRE)
                        osb = o_pool.tile([128, 4, 512], f32, tag="osb")
                        evict(ps[:], bias_sb[:], osb, mj, 0)
            wpx_ctx.__exit__(None, None, None)

    nc.compile()
    return nc


def _get_nc():
    if "nc" not in _CACHE:
        _CACHE["nc"] = _build()
    return _CACHE["nc"]


def _shard(x, factors, bias):
    import ml_dtypes

    bf = ml_dtypes.bfloat16
    x_flat = np.asarray(x, dtype=np.float32).reshape(B * T, DIM).astype(bf)
    factors = np.asarray(factors, dtype=np.float32).astype(bf)
    bias = np.ascontiguousarray(bias, dtype=np.float32)
    in_maps = []
    for c in range(N_CORES):
        tp, dp = c // DP, c % DP
        xc = x_flat[dp * MC:(dp + 1) * MC, :]           # [m, d]
        # -> [mj, p, t, m] with d = t*128+p, m = mj*512+m'
        xh = np.ascontiguousarray(
            xc.T.reshape(NT, 128, NMJ, 512).transpose(2, 1, 0, 3)
        )
        c0 = tp * ECO + dp * ECL
        fc = factors[:, :, c0:c0 + ECL]                 # [r, d, e]
        # -> [ec, p, rp, q, t, e]
        fhc = np.ascontiguousarray(
            fc.reshape(RANK // 2, 2, NT, 128, 2, 128).transpose(4, 3, 0, 1, 2, 5)
        )
        colmap = [tp * ECO + dp * ECL, tp * ECO + dp * ECL + 128,
                  tp * ECO + (1 - dp) * ECL, tp * ECO + (1 - dp) * ECL + 128]
        b4 = np.stack([bias[cm:cm + 128] for cm in colmap], axis=1)  # [128, 4]
        bias_melt = np.ascontiguousarray(
            np.broadcast_to(b4[:, :, None], (128, 4, 512)), dtype=np.float32
        )
        in_maps.append({"xh": xh, "fh": fhc, "bias_melt": bias_melt})
    return in_maps


def _run(in_maps, trace=False, trace_cores=None):
    from concourse.bass_utils import run_bass_kernel_spmd

    nc = _get_nc()
    return run_bass_kernel_spmd(
        nc, in_maps, list(range(N_CORES)), trace=trace, trace_cores=trace_cores
    )


def _assemble(res):
    out = np.empty((B * T, DIM), dtype=np.float32)
    for c in range(N_CORES):
        tp, dp = c // DP, c % DP
        outT = res.results[c]["outT"]
        colmap = [tp * ECO + dp * ECL, tp * ECO + dp * ECL + 128,
                  tp * ECO + (1 - dp) * ECL, tp * ECO + (1 - dp) * ECL + 128]
        for k, cm in enumerate(colmap):
            out[dp * MC:(dp + 1) * MC, cm:cm + 128] = \
                outT[k * 128:(k + 1) * 128, :].T
    return out.reshape(B, T, DIM)


def kernel(x, factors, bias):
    res = _run(_shard(x, factors, bias), trace=False)
    return _assemble(res)


# revision 21
# speedup vs baseline: 1.1446x; 1.0219x over previous
"""Trainium2 kernel for nn_BalancedHamiltonLayer.

Math: out = einsum("btd,rde->bte", x, factors)/sqrt(rank) + bias.
The einsum contracts r as a plain sum, so sum_r (x @ F_r) == x @ (sum_r F_r):
one [16384,2048] @ [2048,2048] GEMM instead of eight.

Distribution over 8 NeuronCores (single SPMD program), DP2 x TP4:
  - core c = (dp, tp) with dp = c//4, tp = c%4 owns output rows
    [dp*8192, (dp+1)*8192) and output cols [tp*512, (tp+1)*512).
  - W reduction is split between the two cores sharing a tp column group:
    core (dp, tp) reduces cols [tp*512 + dp*256, +256) from its factor
    slice (bf16 strided-pair adds on DVE: 3 instructions per 128-col
    chunk instead of a 7-add tree), then one tiny pairwise AllGather
    (1 MB, groups [[0,4],[1,5],[2,6],[3,7]]) swaps the halves.
  - SPMD slot trick: every core computes W_peer = (slot0 + slot1) -
    W_local in fp32 (exact: W_local equals one slot bitwise, so the
    subtraction recovers the other slot's bf16 values exactly). This
    keeps the program identical across cores with no per-core indexing.
  - GEMM in bf16 (fp32 PSUM): phase A = first PRE m-tiles against the
    locally reduced cols (runs in the AllGather's shadow), phase B =
    same m-tiles against peer cols, phase C = remaining m-tiles against
    all four 128-col e-tiles. 1/sqrt(8) scale + bias fused into the
    PSUM eviction (scalar_tensor_tensor on DVE).
  - Each core writes out^T [512e, 8192m] fp32; the host transposes back.

Host side shards/lays out inputs (partition-major swizzle, bf16 cast) and
reassembles the per-core outputs.
"""

import math

import numpy as np

B, T, DIM, RANK = 4, 4096, 2048, 8
N_CORES = 8
DP, TP = 2, 4
MC = (B * T) // DP             # 8192 rows per core
ECO = DIM // TP                # 512 output cols per core
ECL = ECO // DP                # 256 cols reduced locally
NT = DIM // 128                # 16 contraction tiles
NMJ = MC // 512                # 16 m-supertiles per core
PRE = 6                        # m-tiles run split (local cols first)
SCALE = 1.0 / math.sqrt(RANK)

_CACHE = {}


def _build():
    import concourse.bacc as bacc
    import concourse.mybir as mybir
    import concourse.tile as tile

    f32 = mybir.dt.float32
    bf16 = mybir.dt.bfloat16
    add = mybir.AluOpType.add
    mult = mybir.AluOpType.mult
    pair_groups = [[2 * g, 2 * g + 1] for g in range(4)]

    nc = bacc.Bacc(
        "TRN2", target_bir_lowering=False, debug=False, num_devices=N_CORES
    )
    # x^T tiles: d = t*128+p, m_global = dp*8192 + mj*512 + m
    xh = nc.dram_tensor("xh", [NMJ, 128, NT, 512], bf16, kind="ExternalInput").ap()
    # local factor cols: [ec, th, h, p, rp, q, t, e]; r = h*4 + rp*2 + q,
    # d = (th*8 + t)*128 + p, e_global = tp*512 + dp*256 + ec*128 + e.
    # One load unit = (ec, th, h): 1 MB, 8 KB contiguous per partition.
    fh = nc.dram_tensor(
        "fh", [2, 2, 2, 128, 2, 2, NT // 2, 128], bf16, kind="ExternalInput"
    ).ap()
    # bias per (p, slot); slot order [L0, L1, P0, P1]
    bias_mini = nc.dram_tensor("bias_mini", [128, 4], f32,
                               kind="ExternalInput").ap()
    # transposed output: rows = [L0 L1 P0 P1] col chunks, cols = m
    outT = nc.dram_tensor("outT", [ECO, MC], bf16, kind="ExternalOutput").ap()

    with tile.TileContext(nc) as tc:
        with (
            tc.tile_pool(name="const", bufs=1) as const_pool,
            tc.tile_pool(name="dram", bufs=1, space="DRAM") as dram_pool,
            tc.tile_pool(name="wsb", bufs=1) as w_pool,
            tc.tile_pool(name="xa", bufs=6) as x_pool,
        ):
            scope = nc.named_scope
            xsb = [x_pool.tile([128, NT, 512], bf16, tag="x", name=f"x{i}")
                   for i in range(NMJ)]
            # Factor loads + bias enqueue FIRST: the physical DMA queues
            # drain roughly in enqueue order, so anything issued before
            # them adds straight latency to the reduction -> AllGather ->
            # peer-W critical path.
            # Factor loads: 8.4 MB per core dominates the front-end.
            # Load in 8 x 1 MB units (ec, th, h) across both rings with the
            # reduction tree consuming each unit as it lands, so the first
            # W t-blocks unblock matmuls ~15us before the full load ends.
            with tc.tile_pool(name="red", bufs=2) as red_pool:
                bias_sb = const_pool.tile([128, 4], f32)
                nc.scalar.dma_start(bias_sb[:], bias_mini[:])

                # x tiles on gpsimd, staggered behind the factor stream.
                for mj in range(2):
                    with tc.tile_wait_until(0.012 + 0.006 * mj):
                        nc.gpsimd.dma_start(xsb[mj][:], xh[mj])

                wcs, w_pairs = [], []
                for ec in range(2):
                    wcs.append(dram_pool.tile([128, NT, 128], bf16,
                                              name=f"wc{ec}"))
                    w_pairs.append(dram_pool.tile([2, 128, NT, 128], bf16,
                                                  name=f"w_pair{ec}"))

                # Phase R: per (ec, th): two rank-half loads (sync/scalar
                # rings), 5 DVE adds -> wl[:, ec, th-block]. Each chunk's
                # pairwise AllGather fires once both t-halves stored; x
                # triggers interleave so AG1's input-semaphore wait (which
                # blocks the gpsimd sequencer) never delays an x load.
                wl = w_pool.tile([128, 2, NT, 128], bf16)
                NH = NT // 2
                for ec in range(2):
                    for th in range(2):
                        fta = red_pool.tile([128, 2, 2, NH, 128], bf16,
                                            tag="fra", name=f"fa{ec}{th}")
                        nc.sync.dma_start(fta[:], fh[ec, th, 0])
                        ftb = red_pool.tile([128, 2, 2, NH, 128], bf16,
                                            tag="frb", name=f"fb{ec}{th}")
                        nc.scalar.dma_start(ftb[:], fh[ec, th, 1])
                        with scope(f"red{ec}{th}"):
                            s2 = red_pool.tile([128, 2, NH, 128], bf16,
                                               tag="s2")
                            for h, ft in ((0, fta), (1, ftb)):
                                s1 = red_pool.tile([128, 2, NH, 128], bf16,
                                                   tag=f"s1{h}")
                                nc.vector.tensor_add(s1[:], ft[:, 0], ft[:, 1])
                                nc.vector.tensor_add(s2[:, h], s1[:, 0],
                                                     s1[:, 1])
                            nc.vector.tensor_add(
                                wl[:, ec, th * NH:(th + 1) * NH], s2[:, 0],
                                s2[:, 1])
                    nc.sync.dma_start(wcs[ec][:], wl[:, ec])
                    with scope(f"ag{ec}"):
                        nc.gpsimd.collective_compute(
                            "AllGather", mybir.AluOpType.bypass,
                            ins=[wcs[ec].opt()],
                            outs=[w_pairs[ec].opt()],
                            replica_groups=pair_groups,
                        )
                    for mj in (2, 3) if ec == 0 else (4, 5):
                        with tc.tile_wait_until(0.012 + 0.0065 * mj):
                            nc.gpsimd.dma_start(xsb[mj][:], xh[mj])

            # Peer W inputs: all on the scalar ring so the (late-firing)
            # triggers never block evict stores, which all go on sync.
            # Separate pool opened after red closes, so the factor tiles
            # and the peer-W tiles never need SBUF space simultaneously.
            wpx_ctx = tc.tile_pool(name="wpx", bufs=1)
            wpx_pool = wpx_ctx.__enter__()
            wp = [wpx_pool.tile([128, 2, NT, 128], bf16, name=f"wp{ec}")
                  for ec in range(2)]
            for ec in range(2):
                for s in range(2):
                    nc.scalar.dma_start(wp[ec][:, s], w_pairs[ec][s])
            wsum = wpx_pool.tile([128, NT, 128], f32)
            wpeer = wpx_pool.tile([128, 2, NT, 128], bf16)

            # slot0 + slot1 - W_local == the peer chunk, exactly (W_local
            # is bitwise one of the slots). On gpsimd: slower than DVE but
            # entirely off the DVE eviction stream, so the scheduler cannot
            # order it ahead of phase-A evictions (psum backpressure).
            for ec in range(2):
                with scope(f"peer{ec}"):
                    nc.gpsimd.tensor_add(wsum[:], wp[ec][:, 0], wp[ec][:, 1])
                    nc.gpsimd.tensor_sub(wpeer[:, ec], wsum[:], wl[:, ec])

            def sweep(ps_slice, w_ec, xt):
                for t in range(NT):
                    nc.tensor.matmul(
                        ps_slice,
                        w_ec[:, t, :],
                        xt[:, t, :],
                        start=(t == 0),
                        stop=(t == NT - 1),
                    )

            # Evictions on the otherwise-idle ACT engine (Identity with
            # per-partition bias + scale), keeping the DVE stream free for
            # the reduction adds so the scheduler cannot interleave them.
            ident = mybir.ActivationFunctionType.Identity

            def evict(ps_slice, osb, mj, r0):
                for k in range(osb.shape[1]):
                    nc.scalar.activation(
                        osb[:, k], ps_slice[:, k], ident,
                        bias=bias_sb[:, r0 + k:r0 + k + 1], scale=SCALE,
                    )
                    nc.sync.dma_start(
                        outT[(r0 + k) * 128:(r0 + k + 1) * 128,
                             mj * 512:(mj + 1) * 512],
                        osb[:, k],
                    )

            def prefetch(i):
                if i < NMJ:
                    nc.gpsimd.dma_start(xsb[i][:], xh[i])

            with (
                tc.tile_pool(name="osb", bufs=2) as o_pool,
                tc.tile_pool(name="ps", bufs=2, space="PSUM") as p_pool,
            ):
                # Phase A: first PRE m-tiles x local cols (AllGather shadow).
                # peer_ops(0) is slotted into the DVE stream near the end of
                # A: late enough not to head-of-line block A's evictions
                # behind the AllGather, early enough to unblock phase B0.
                for mj in range(PRE):
                    with scope(f"gA{mj}"):
                        ps = p_pool.tile([128, 4, 512], f32, tag="ps")
                        sweep(ps[:, 0, :], wl[:, 0], xsb[mj])
                        sweep(ps[:, 1, :], wl[:, 1], xsb[mj])
                        osb = o_pool.tile([128, 2, 512], bf16, tag="osb")
                        evict(ps[:, 0:2], osb, mj, 0)
                # Phase B0: same m-tiles x first peer chunk (single-sweep
                # units; the L/P column blocks are independent outputs, so
                # the two peer chunks can land as separate passes).
                for mj in range(PRE):
                    with scope(f"gB0_{mj}"):
                        ps = p_pool.tile([128, 4, 512], f32, tag="ps")
                        sweep(ps[:, 0, :], wpeer[:, 0], xsb[mj])
                        osb = o_pool.tile([128, 1, 512], bf16, tag="osb")
                        evict(ps[:, 0:1], osb, mj, 2)
                # Phase B1: same m-tiles x second peer chunk.
                for mj in range(PRE):
                    with scope(f"gB1_{mj}"):
                        ps = p_pool.tile([128, 4, 512], f32, tag="ps")
                        sweep(ps[:, 0, :], wpeer[:, 1], xsb[mj])
                        prefetch(PRE + mj)
                        osb = o_pool.tile([128, 1, 512], bf16, tag="osb")
                        evict(ps[:, 0:1], osb, mj, 3)
                # Phase C: remaining m-tiles x all four e-tiles.
                for mj in range(PRE, NMJ):
                    with scope(f"gC{mj}"):
                        ps = p_pool.tile([128, 4, 512], f32, tag="ps")
                        sweep(ps[:, 0, :], wl[:, 0], xsb[mj])
                        sweep(ps[:, 1, :], wl[:, 1], xsb[mj])
                        sweep(ps[:, 2, :], wpeer[:, 0], xsb[mj])
                        sweep(ps[:, 3, :], wpeer[:, 1], xsb[mj])
                        prefetch(mj + PRE)
                        osb = o_pool.tile([128, 4, 512], bf16, tag="osb")
                        evict(ps[:], osb, mj, 0)
            wpx_ctx.__exit__(None, None, None)

    nc.compile()
    return nc


def _get_nc():
    if "nc" not in _CACHE:
        _CACHE["nc"] = _build()
    return _CACHE["nc"]


def _shard(x, factors, bias):
    import ml_dtypes

    bf = ml_dtypes.bfloat16
    x_flat = np.asarray(x, dtype=np.float32).reshape(B * T, DIM).astype(bf)
    factors = np.asarray(factors, dtype=np.float32).astype(bf)
    bias = np.ascontiguousarray(bias, dtype=np.float32)
    in_maps = []
    for c in range(N_CORES):
        tp, dp = c // DP, c % DP
        xc = x_flat[dp * MC:(dp + 1) * MC, :]           # [m, d]
        # -> [mj, p, t, m] with d = t*128+p, m = mj*512+m'
        xh = np.ascontiguousarray(
            xc.T.reshape(NT, 128, NMJ, 512).transpose(2, 1, 0, 3)
        )
        c0 = tp * ECO + dp * ECL
        fc = factors[:, :, c0:c0 + ECL]                 # [r, d, e]
        # -> [ec, th, h, p, rp, q, t, e]; r = h*4+rp*2+q, d = (th*8+t)*128+p
        fhc = np.ascontiguousarray(
            fc.reshape(2, 2, 2, 2, NT // 2, 128, 2, 128)
              .transpose(6, 3, 0, 5, 1, 2, 4, 7)
        )
        colmap = [tp * ECO + dp * ECL, tp * ECO + dp * ECL + 128,
                  tp * ECO + (1 - dp) * ECL, tp * ECO + (1 - dp) * ECL + 128]
        b4 = np.ascontiguousarray(
            np.stack([bias[cm:cm + 128] for cm in colmap], axis=1),
            dtype=np.float32)
        in_maps.append({"xh": xh, "fh": fhc, "bias_mini": b4})
    return in_maps


def _run(in_maps, trace=False, trace_cores=None):
    from concourse.bass_utils import run_bass_kernel_spmd

    nc = _get_nc()
    return run_bass_kernel_spmd(
        nc, in_maps, list(range(N_CORES)), trace=trace, trace_cores=trace_cores
    )


def _assemble(res):
    out = np.empty((B * T, DIM), dtype=np.float32)
    for c in range(N_CORES):
        tp, dp = c // DP, c % DP
        outT = res.results[c]["outT"]
        colmap = [tp * ECO + dp * ECL, tp * ECO + dp * ECL + 128,
                  tp * ECO + (1 - dp) * ECL, tp * ECO + (1 - dp) * ECL + 128]
        for k, cm in enumerate(colmap):
            out[dp * MC:(dp + 1) * MC, cm:cm + 128] = \
                outT[k * 128:(k + 1) * 128, :].T.astype(np.float32)
    return out.reshape(B, T, DIM)


def kernel(x, factors, bias):
    res = _run(_shard(x, factors, bias), trace=False)
    return _assemble(res)
